# revision 1
# baseline (speedup 1.0000x reference)
"""Trainium2 Bass kernel for nn_ConsistentSelfAttentionProcessor.

Reference computation (per frame-set of NUM_FRAMES=4 frames):
    q,k,v = hs@Wq+bq, hs@Wk+bk, hs@Wv+bv          # [BF,S,D]
    per head: K_comb = [K(frame0_of_set); K(own)]  # 2S keys
    out = softmax(q@K_comb^T/sqrt(hd)) @ V_comb @ Wo + bo + hs

Sharding: 8 cores = 2 frame-sets x 4 head-groups of 5 heads.
Each core computes a partial output  attn(set, heads_g) @ Wo[rows_g]  in bf16;
the host sums the 4 per-set partials in fp32 and adds bo + residual.

Frame 0 of each set attends to [K0;K0] which equals softmax over K0 alone,
so frame 0 uses 1024 keys instead of 2048.

Softmax uses no max subtraction: scores*0.125 is bounded (~|3|) for these
inputs, so exp is safe in fp32. The softmax denominator comes for free from a
ones-column appended to V (U_T row 64 = sum(exp)).
"""

import sys
from contextlib import ExitStack

import numpy as np

sys.path.insert(0, "/opt/trn_rl_repo")

import ml_dtypes  # noqa: E402

import concourse.bass as bass  # noqa: E402
import concourse.mybir as mybir  # noqa: E402
import concourse.tile as tile  # noqa: E402
from concourse import bacc, bass_utils  # noqa: E402
from concourse.masks import make_identity  # noqa: E402

BF16 = mybir.dt.bfloat16
F32 = mybir.dt.float32
NPBF16 = ml_dtypes.bfloat16

NUM_FRAMES = 4
HEADS = 20
BF, S, D = 8, 1024, 1280
HD = 64  # head dim
B = BF // NUM_FRAMES  # 2 frame sets
N_CORES = 8
GROUPS = 4  # head groups per set
HG = HEADS // GROUPS  # 5 heads per group
C = HG * HD  # 320 columns per group
N_SET = NUM_FRAMES * S  # 4096 rows per set
SCALE = 1.0 / np.sqrt(HD)  # 0.125

P = 128
KC_D = D // P  # 10 contraction chunks for projections
TC_N = N_SET // P  # 32 token chunks per set
QH = 2  # q halves of 512 per frame


def build_kernel_body(ctx: ExitStack, tc: tile.TileContext, xt, wqkv, wo, bqkv, out):
    """Emit the per-core program.

    xt:   [D, N_SET]      bf16  (X^T for this set)
    wqkv: [D, 3*C]        bf16  (columns: Wq_g | Wk_g | Wv_g)
    wo:   [3*P, D]        bf16  (rows 0..C-1 = Wo[group rows]; rest zero pad)
    bqkv: [3*C]           f32
    out:  [N_SET, D]      bf16  (partial output, unsummed, no bo/residual)
    """
    nc = tc.nc

    const = ctx.enter_context(tc.tile_pool(name="const", bufs=1))
    persist = ctx.enter_context(tc.tile_pool(name="persist", bufs=1))
    work = ctx.enter_context(tc.tile_pool(name="work", bufs=3))
    psum = ctx.enter_context(tc.tile_pool(name="psum", bufs=1, space="PSUM"))

    # ---- constants ----------------------------------------------------------
    ident = const.tile([P, P], BF16, tag="ident")
    make_identity(nc, ident)
    ones = const.tile([P, P], F32, tag="ones")
    nc.gpsimd.memset(ones, 1.0)

    wqkv_sb = const.tile([P, KC_D, 3 * C], BF16, tag="wqkv")
    nc.sync.dma_start(wqkv_sb, wqkv.rearrange("(c p) n -> p c n", p=P))
    wo_sb = const.tile([P, 3, D], BF16, tag="wo")
    nc.sync.dma_start(wo_sb, wo.rearrange("(c p) n -> p c n", p=P))
    bqkv_sb = const.tile([1, 3 * C], F32, tag="bqkv")
    nc.sync.dma_start(bqkv_sb, bqkv[None, :])

    # broadcast biases across partitions once: bias_bc[p, j] = bqkv[j]
    bias_bc = const.tile([P, 3 * C], F32, tag="bias_bc")
    bps = psum.tile([P, 3 * C], F32, tag="A", bufs=2)
    nc.tensor.matmul(bps[:, 0:512], ones[0:1, :], bqkv_sb[:, 0:512])
    nc.tensor.matmul(bps[:, 512:960], ones[0:1, :], bqkv_sb[:, 512:960])
    nc.vector.tensor_copy(bias_bc, bps)

    # ---- persistent intermediates ------------------------------------------
    # Q^T/K^T, head-transposed: chunk h//2 holds head pair, base (h%2)*64.
    # chunks 0-2: q-heads, 3-5: k-heads (halves of chunks 2 and 5 unused).
    qkt = persist.tile([P, 6, N_SET], BF16, tag="qkt")
    # V rows with a ones column per head: [tokens, head, 65]
    vsb = persist.tile([P, TC_N, HG, HD + 1], BF16, tag="vsb")
    nc.gpsimd.memset(vsb[:, :, :, HD], 1.0)
    # attn^T for O-proj, one tensor per frame so O-proj(f) only depends on
    # frame f's attention: chunk c holds heads (2c, 2c+1); chunk 2 half unused
    atn_f = [
        persist.tile([P, 3, S], BF16, tag=f"atn{f}", name=f"atn{f}")
        for f in range(NUM_FRAMES)
    ]
    for f in range(NUM_FRAMES):
        nc.gpsimd.memset(atn_f[f][64:128, 2, :], 0.0)

    # ---- phase 1: QKV projections ------------------------------------------
    for t in range(TC_N):
        xcol = work.tile([P, KC_D, P], BF16, tag="xcol")
        nc.sync.dma_start(
            xcol, xt[:, t * P : (t + 1) * P].rearrange("(c p) n -> p c n", p=P)
        )
        pq = psum.tile([P, 3 * C], F32, tag="A", bufs=2)
        for kc in range(KC_D):
            st, sp = kc == 0, kc == KC_D - 1
            nc.tensor.matmul(
                pq[:, 0:512], xcol[:, kc], wqkv_sb[:, kc, 0:512], start=st, stop=sp
            )
            nc.tensor.matmul(
                pq[:, 512:960], xcol[:, kc], wqkv_sb[:, kc, 512:960], start=st, stop=sp
            )
        # V part: bias add + split per head into vsb
        nc.vector.tensor_tensor(
            vsb[:, t, :, 0:HD],
            pq[:, 2 * C : 3 * C].rearrange("p (h d) -> p h d", d=HD),
            bias_bc[:, 2 * C : 3 * C].rearrange("p (h d) -> p h d", d=HD),
            mybir.AluOpType.add,
        )
        # QK part: bias add + cast, then PE-transpose into qkt
        rows = work.tile([P, 2 * C], BF16, tag="rows")
        nc.vector.tensor_tensor(
            rows, pq[:, 0 : 2 * C], bias_bc[:, 0 : 2 * C], mybir.AluOpType.add
        )
        # 6 transposes: (q0q1)(q2q3)(q4)(k0k1)(k2k3)(k4)
        for ch in range(6):
            width = HD if ch in (2, 5) else P
            src = rows[:, ch * P : ch * P + width] if ch < 3 else rows[
                :, C + (ch - 3) * P : C + (ch - 3) * P + width
            ]
            tp = psum.tile([P, P], BF16, tag="C", bufs=2)
            nc.tensor.transpose(tp[0:width, :], src, ident)
            nc.vector.tensor_copy(qkt[0:width, ch, t * P : (t + 1) * P], tp[0:width, :])

    # ---- phase 2+3: attention, O-proj per frame -----------------------------
    for f in range(NUM_FRAMES):
        qoff = f * S
        nkc = 8 if f == 0 else 16  # frame 0: ref==own, dedup
        for h in range(HG):
            b = (h % 2) * HD  # partition base for this head
            qch = h // 2
            kch = 3 + h // 2
            ut = psum.tile([P, S], F32, tag="ut", bufs=1)
            for kc in range(nkc):
                # key token position: first 8 chunks ref frame, rest own frame
                ktok = kc * P if kc < 8 else qoff + (kc - 8) * P
                sc = psum.tile([P, S], F32, tag="A", bufs=2)
                for q in range(QH):
                    nc.tensor.matmul(
                        sc[:, q * 512 : (q + 1) * 512],
                        qkt[b : b + HD, kch, ktok : ktok + P],
                        qkt[b : b + HD, qch, qoff + q * 512 : qoff + (q + 1) * 512],
                    )
                ex = work.tile([P, S], BF16, tag="ex")
                nc.scalar.activation(
                    ex, sc, mybir.ActivationFunctionType.Exp, scale=SCALE
                )
                for q in range(QH):
                    nc.tensor.matmul(
                        ut[0 : HD + 1, q * 512 : (q + 1) * 512],
                        vsb[:, ktok // P, h, :],
                        ex[:, q * 512 : (q + 1) * 512],
                        start=(kc == 0),
                        stop=(kc == nkc - 1),
                    )
            # normalize: attn^T = ut[0:64] / bcast(ut[64]).  Keep PE out of
            # this tail: DVE copies s to SBUF, idle GpSimd broadcasts it
            # across partitions, DVE divides (single PSUM operand rule ok).
            rc = work.tile([HD + 1, S], F32, tag="rc", bufs=2)
            nc.vector.reciprocal(rc[HD : HD + 1, :], ut[HD : HD + 1, :])
            for q in range(QH):
                qs = slice(q * 512, (q + 1) * 512)
                bcp = psum.tile([HD, 512], F32, tag="C", bufs=2)
                nc.tensor.matmul(bcp, ones[HD : HD + 1, 0:HD], rc[HD : HD + 1, qs])
                # DVE can read only one PSUM operand per op: stage via SBUF
                bc = work.tile([HD, 512], F32, tag="bcs", bufs=2)
                nc.vector.tensor_copy(bc, bcp)
                if h % 2 == 0:
                    nc.vector.tensor_tensor(
                        atn_f[f][0:HD, h // 2, q * 512 : (q + 1) * 512],
                        ut[0:HD, qs],
                        bc,
                        mybir.AluOpType.mult,
                    )
                else:
                    # result must land at partitions 64-127: mult to a base-0
                    # tmp, then PE-copy shifts partitions
                    tm = work.tile([HD, 512], BF16, tag="tm", bufs=2)
                    nc.vector.tensor_tensor(tm, ut[0:HD, qs], bc, mybir.AluOpType.mult)
                    pc = psum.tile([P, 512], F32, tag="C", bufs=2)
                    nc.tensor.matmul(pc[HD:P, :], ident[0:HD, 0:HD], tm)
                    nc.vector.tensor_copy(
                        atn_f[f][HD:P, h // 2, q * 512 : (q + 1) * 512],
                        pc[HD:P, :],
                    )
        # O-proj for this frame's 8 token chunks (fills ACT-bound gaps of the
        # next frame's attention on PE)
        for tl in range(S // P):
            t = f * (S // P) + tl
            ou = work.tile([P, D], BF16, tag="ou")
            for n3, nw in ((0, 512), (1, 512), (2, 256)):
                po = psum.tile([P, 512], F32, tag="A", bufs=2)
                for kc in range(3):
                    nc.tensor.matmul(
                        po[:, 0:nw],
                        atn_f[f][:, kc, tl * P : (tl + 1) * P],
                        wo_sb[:, kc, n3 * 512 : n3 * 512 + nw],
                        start=(kc == 0),
                        stop=(kc == 2),
                    )
                nc.vector.tensor_copy(ou[:, n3 * 512 : n3 * 512 + nw], po[:, 0:nw])
            nc.sync.dma_start(out[t * P : (t + 1) * P, :], ou)


def build_program():
    from concourse.bass_interp import get_hw_module

    nc = bacc.Bacc(
        "TRN2",
        target_bir_lowering=False,
        debug=False,
        enable_asserts=False,
        num_devices=N_CORES,
    )
    xt = nc.dram_tensor("xt", [D, N_SET], BF16, kind="ExternalInput").ap()
    wqkv = nc.dram_tensor("wqkv", [D, 3 * C], BF16, kind="ExternalInput").ap()
    wo = nc.dram_tensor("wo", [3 * P, D], BF16, kind="ExternalInput").ap()
    bqkv = nc.dram_tensor("bqkv", [3 * C], F32, kind="ExternalInput").ap()
    out = nc.dram_tensor("out", [N_SET, D], BF16, kind="ExternalOutput").ap()
    with tile.TileContext(nc) as tc:
        with ExitStack() as ctx:
            build_kernel_body(ctx, tc, xt, wqkv, wo, bqkv, out)
    nc.finalize()
    nc.m = get_hw_module(nc.m)
    return nc


def make_in_maps(hidden_states, Wq, Wk, Wv, bq, bk, bv):
    """Per-core inputs. Core c = set (c//4), head group (c%4)."""
    hs = np.asarray(hidden_states, np.float32).reshape(BF, S, D)
    in_maps = []
    xts = []
    for s in range(B):
        x = hs[s * NUM_FRAMES : (s + 1) * NUM_FRAMES].reshape(N_SET, D)
        xts.append(np.ascontiguousarray(x.T).astype(NPBF16))
    for c in range(N_CORES):
        s, g = c // GROUPS, c % GROUPS
        cols = slice(g * C, (g + 1) * C)
        wqkv = np.concatenate(
            [np.asarray(W, np.float32)[:, cols] for W in (Wq, Wk, Wv)], axis=1
        ).astype(NPBF16)
        bqkv = np.concatenate(
            [np.asarray(bb, np.float32)[cols] for bb in (bq, bk, bv)]
        ).astype(np.float32)
        in_maps.append(
            {"xt": xts[s], "wqkv": wqkv, "bqkv": bqkv}
        )
    return in_maps


def make_wo_pad(Wo, g):
    wo_g = np.asarray(Wo, np.float32)[g * C : (g + 1) * C, :]  # [320, 1280]
    wo_pad = np.zeros((3 * P, D), np.float32)
    wo_pad[:C] = wo_g
    return wo_pad.astype(NPBF16)


_PROGRAM = None


def kernel(hidden_states, Wq, Wk, Wv, Wo, bq, bk, bv, bo):
    global _PROGRAM
    if _PROGRAM is None:
        _PROGRAM = build_program()
    nc = _PROGRAM

    in_maps = make_in_maps(hidden_states, Wq, Wk, Wv, bq, bk, bv)
    for c in range(N_CORES):
        in_maps[c]["wo"] = make_wo_pad(Wo, c % GROUPS)

    res = bass_utils.run_bass_kernel_spmd(nc, in_maps, core_ids=list(range(N_CORES)))
    hs = np.asarray(hidden_states, np.float32)
    bo = np.asarray(bo, np.float32)
    out = np.empty((BF, S, D), np.float32)
    for s in range(B):
        acc = np.zeros((N_SET, D), np.float32)
        for g in range(GROUPS):
            acc += np.asarray(res.results[s * GROUPS + g]["out"], np.float32)
        out[s * NUM_FRAMES : (s + 1) * NUM_FRAMES] = (
            acc.reshape(NUM_FRAMES, S, D)
            + bo[None, None, :]
            + hs[s * NUM_FRAMES : (s + 1) * NUM_FRAMES]
        )
    return out



# revision 7
# speedup vs baseline: 1.3752x; 1.3752x over previous
"""Trainium2 Bass kernel for nn_ConsistentSelfAttentionProcessor (v2, fp8).

Reference computation (per frame-set of NUM_FRAMES=4 frames):
    q,k,v = hs@Wq+bq, hs@Wk+bk, hs@Wv+bv          # [BF,S,D]
    per head: K_comb = [K(frame0_of_set); K(own)]  # 2S keys
    out = softmax(q@K_comb^T/sqrt(hd)) @ V_comb @ Wo + bo + hs

Sharding: 8 cores = 2 frame-sets x 4 head-groups of 5 heads.
Core computes a partial  attn(set, heads_g) @ Wo[rows_g]  in bf16 (x4096);
host sums the 4 per-set partials, divides by 4096, adds bo + bv@Wo + residual.

v2 design (vs bf16 v1):
 - everything on the PE in fp8e4 (weights host-scaled x64 q/k/v, x16 Wo so
   values sit in fp8 normal range; scales divided out on the host)
 - q^T/k^T produced directly transposed (W stationary, x^T moving): no PE
   transposes.  2 heads packed per 128-partition chunk (parity rows).
 - DoubleRow fp8 matmuls (2x128 contraction per pass) for projections and AV.
 - V stationary blocks are [v(64)|ones(64)] (even heads) / [ones|v] (odd), so
   the AV matmul emits 64 broadcast copies of the softmax denominator next to
   the 64 attn rows: normalization needs no PE/DMA broadcast.
 - softmax exp split between ACT (hardware Exp->fp8) and DVE (Schraudolph
   int8 bit-trick writing fp8e4 bit patterns directly).
 - denominator reciprocal via int32 bit-trick on DVE (no 6.5us RECIPROCAL),
   row-shifted to the attn partitions with a tiny SBUF->SBUF DMA.
 - scores software-pipelined one iteration ahead of AV so the PE never waits
   on exp.

Frame 0 of each set attends to [K0;K0] == softmax over K0: uses 4 key-chunk
pairs instead of 8.
"""

import sys
from contextlib import ExitStack

import numpy as np

sys.path.insert(0, "/opt/trn_rl_repo")

import ml_dtypes  # noqa: E402

import concourse.bass as bass  # noqa: E402
import concourse.mybir as mybir  # noqa: E402
import concourse.tile as tile  # noqa: E402
from concourse import bacc, bass_utils  # noqa: E402

FP8 = mybir.dt.float8e4
BF16 = mybir.dt.bfloat16
F32 = mybir.dt.float32
I8 = mybir.dt.int8
I32 = mybir.dt.int32
NPFP8 = ml_dtypes.float8_e4m3
NPBF16 = ml_dtypes.bfloat16
DR = mybir.MatmulPerfMode.DoubleRow
EXP = mybir.ActivationFunctionType.Exp
IDENT = mybir.ActivationFunctionType.Identity
MULT = mybir.AluOpType.mult
ADD = mybir.AluOpType.add

NUM_FRAMES = 4
HEADS = 20
BF, S, D = 8, 1024, 1280
HD = 64
B = BF // NUM_FRAMES
N_SET = NUM_FRAMES * S  # 4096
N_CORES = 8
GROUPS = 4
HG = HEADS // GROUPS  # 5
C = HG * HD  # 320
P = 128
KC_D = D // P  # 10

# host-side scalings (divided out on host)
WQK_SC = 32.0
WV_SC = 32.0
WO_SC = 16.0
ATN_SC = 8.0  # applied in the normalize mult
OUT_DIV = WV_SC * ATN_SC * WO_SC  # 4096

SCALE_EXP = 0.125 / (WQK_SC * WQK_SC)  # exp(score_dev * SCALE_EXP)
SCH_A = (8.0 / np.log(2.0)) * SCALE_EXP
SCH_B = 55.5
RMAGIC = 0x7EF0FFFF

# qkt chunk map: 6 chunks of [low(0:64) | high(64:128)] rows
#   ch0=[q0|q1] ch1=[q2|q3] ch2=[k0|k1] ch3=[k2|k3] ch4=[q4|-] ch5=[k4|-]
QCH = [0, 0, 1, 1, 4]
KCH = [2, 2, 3, 3, 5]
HROW = [0, 64, 0, 64, 0]  # base row (parity) of head h


def build_kernel_body(ctx: ExitStack, tc: tile.TileContext, xt, wqk, wv, wo,
                      bqk, out):
    """Per-core program.

    xt:   [D, N_SET]          fp8  (X^T for this set)
    wqk:  [KC_D, P, 6, P]     fp8  (transposed-proj stationaries, x64)
    wv:   [KC_D, P, C]        fp8  (x64)
    wo:   [3, P, D]           fp8  (O-proj moving, x16; ch2 rows 64:128 zero)
    bqk:  [P, 6]              f32  (per-partition q/k biases, x64)
    out:  [N_SET, D]          bf16 (partial output x4096, unsummed)
    """
    nc = tc.nc

    const = ctx.enter_context(tc.tile_pool(name="const", bufs=1))
    persist = ctx.enter_context(tc.tile_pool(name="persist", bufs=1))
    work = ctx.enter_context(tc.tile_pool(name="work", bufs=3))
    psA = ctx.enter_context(tc.tile_pool(name="psA", bufs=2, space="PSUM"))
    psU = ctx.enter_context(tc.tile_pool(name="psU", bufs=2, space="PSUM"))
    psO = ctx.enter_context(tc.tile_pool(name="psO", bufs=2, space="PSUM"))

    # ---- constant loads -----------------------------------------------------
    xt_sb = const.tile([P, KC_D, N_SET], FP8, tag="xt")
    for q in range(4):
        nc.sync.dma_start(
            xt_sb[:, :, q * 1024:(q + 1) * 1024],
            xt[:, q * 1024:(q + 1) * 1024].rearrange("(c p) n -> p c n", p=P),
        )
    wqk_sb = const.tile([P, KC_D, 6, P], FP8, tag="wqk")
    nc.sync.dma_start(wqk_sb, wqk.rearrange("c p g m -> p c g m"))
    wv_sb = const.tile([P, KC_D, C], FP8, tag="wv")
    nc.sync.dma_start(wv_sb, wv.rearrange("c p n -> p c n"))
    wo_sb = const.tile([P, 3, D], FP8, tag="wo")
    nc.sync.dma_start(wo_sb, wo.rearrange("c p n -> p c n"))
    bqk_sb = const.tile([P, 6], F32, tag="bqk")
    nc.sync.dma_start(bqk_sb, bqk)

    # ---- persistent intermediates ------------------------------------------
    qkt = persist.tile([P, 6, N_SET], FP8, tag="qkt")
    # vsb blocks per head: even [v|ones], odd [ones|v]
    vsb = persist.tile([P, N_SET // P, HG, P], FP8, tag="vsb")
    nc.gpsimd.memset(vsb[:, :, 0::2, HD:P], 1.0)
    nc.gpsimd.memset(vsb[:, :, 1::2, 0:HD], 1.0)
    atn_f = [
        persist.tile([P, 3, S], FP8, tag=f"atn{f}", name=f"atn{f}")
        for f in range(NUM_FRAMES)
    ]

    copy_tog = [0]

    def psum_to_sbuf(dst, src, bias=None):
        """Alternate psum->sbuf copies between ACT and DVE."""
        copy_tog[0] ^= 1
        if copy_tog[0]:
            if bias is not None:
                nc.scalar.activation(dst, src, IDENT, bias=bias, scale=1.0)
            else:
                nc.scalar.copy(dst, src)
        else:
            if bias is not None:
                nc.vector.tensor_scalar(dst, src, bias, None, ADD)
            else:
                nc.vector.tensor_copy(dst, src)

    # ---- phase 1: projections (token-major for early attention start) ------
    for ts in range(8):  # 512-token slabs
        tok = slice(ts * 512, (ts + 1) * 512)
        for g in range(6):
            pq = psO.tile([P, 512], F32, tag="O", bufs=2)
            for i in range(KC_D // 2):
                nc.tensor.matmul(
                    pq, wqk_sb[:, 2 * i:2 * i + 2, g, :],
                    xt_sb[:, 2 * i:2 * i + 2, tok],
                    perf_mode=DR, start=(i == 0), stop=(i == KC_D // 2 - 1),
                )
            rows = slice(0, P) if g < 4 else slice(0, HD)
            psum_to_sbuf(qkt[rows, g, tok], pq[rows, :],
                         bias=bqk_sb[rows, g:g + 1])
        for tl in range(4):  # 128-token chunks for V
            t = ts * 4 + tl
            pv = psO.tile([P, 512], F32, tag="O", bufs=2)
            for i in range(KC_D // 2):
                nc.tensor.matmul(
                    pv[:, 0:C],
                    xt_sb[:, 2 * i:2 * i + 2, t * P:(t + 1) * P],
                    wv_sb[:, 2 * i:2 * i + 2, :],
                    perf_mode=DR, start=(i == 0), stop=(i == KC_D // 2 - 1),
                )
            pvh = pv[:, 0:C].rearrange("p (h d) -> p h d", d=HD)
            psum_to_sbuf(vsb[:, t, 0::2, 0:HD], pvh[:, 0::2, :])
            psum_to_sbuf(vsb[:, t, 1::2, HD:P], pvh[:, 1::2, :])

    # ---- phase 2/3: attention + O-proj, one flat software pipeline ---------
    # iteration = (f, h, qh, kp): kp indexes key-chunk pairs (256 keys each)
    iters = []
    for f in range(NUM_FRAMES):
        nkp = 4 if f == 0 else 8
        for h in range(HG):
            for qh in range(2):
                for kp in range(nkp):
                    iters.append((f, h, qh, kp, kp == 0, kp == nkp - 1))

    oproj_queue = []  # (f, t) token chunks ready for O-proj
    exp_tog = [0]

    def emit_scores_exp(f, h, qh, kp):
        """scores for pair kp -> exp -> ex tile; returns ex."""
        b0 = HROW[h]
        qspan = slice(f * S + qh * 512, f * S + (qh + 1) * 512)
        sc = psA.tile([P, 2, 512], F32, tag="A", bufs=2)
        for c in range(2):
            if kp < 4:
                kt = (2 * kp + c) * P  # ref frame (set tokens 0:1024)
            else:
                kt = f * S + (2 * (kp - 4) + c) * P
            nc.tensor.matmul(
                sc[:, c, :],
                qkt[b0:b0 + HD, KCH[h], kt:kt + P],
                qkt[b0:b0 + HD, QCH[h], qspan],
            )
        ex = work.tile([P, 2, 512], FP8, tag="ex", bufs=4)
        exp_tog[0] = (exp_tog[0] + 1) % 8
        if exp_tog[0] < 5:
            nc.scalar.activation(ex, sc, EXP, scale=SCALE_EXP)
        else:
            nc.vector.tensor_scalar(ex.bitcast(I8), sc, SCH_A, SCH_B,
                                    MULT, ADD)
        return ex

    def emit_av(ut, f, h, kp, ex, first, last):
        if kp < 4:
            tp = 2 * kp
        else:
            tp = f * 8 + 2 * (kp - 4)
        nc.tensor.matmul(
            ut, vsb[:, tp:tp + 2, h, :], ex,
            perf_mode=DR, start=first, stop=last,
        )

    def emit_norm(ut, f, h, qh):
        """atn[rows, ch, qh*512:] = (ut[rows]*4) * (1/denom)."""
        if h % 2 == 0:
            arows, drows = slice(0, HD), slice(HD, P)
        else:
            arows, drows = slice(HD, P), slice(0, HD)
        rc1 = work.tile([P, 512], F32, tag="rc1", bufs=3)
        nc.vector.tensor_scalar(rc1[drows, :].bitcast(I32),
                                ut[drows, :].bitcast(I32),
                                -1, RMAGIC, MULT, ADD)
        rc2 = work.tile([P, 512], F32, tag="rc2", bufs=3)
        nc.sync.dma_start(rc2[arows, :], rc1[drows, :])
        nc.vector.scalar_tensor_tensor(
            atn_f[f][arows, h // 2, qh * 512:(qh + 1) * 512],
            ut[arows, :], ATN_SC, rc2[arows, :], MULT, MULT,
        )

    ou_state = {}  # t -> ou tile

    def emit_oproj_group(f, tl, g):
        """One O-proj psum group: out[t, g*512:...] for token chunk tl."""
        t = f * 8 + tl
        n0 = g * 512
        nw = 512 if g < 2 else 256
        po = psO.tile([P, 512], F32, tag="O", bufs=2)
        nc.tensor.matmul(
            po[:, 0:nw], atn_f[f][:, 0:2, tl * P:(tl + 1) * P],
            wo_sb[:, 0:2, n0:n0 + nw],
            perf_mode=DR, start=True, stop=False,
        )
        nc.tensor.matmul(
            po[:, 0:nw], atn_f[f][0:HD, 2, tl * P:(tl + 1) * P],
            wo_sb[0:HD, 2, n0:n0 + nw],
            start=False, stop=True,
        )
        if g == 0:
            ou_state[t] = work.tile([P, D], BF16, tag="ou", bufs=3,
                                    name=f"ou{t}")
        psum_to_sbuf(ou_state[t][:, n0:n0 + nw], po[:, 0:nw])
        if g == 2:
            nc.sync.dma_start(out[t * P:(t + 1) * P, :], ou_state.pop(t))

    # flat pipeline: scores(i) emitted one iteration ahead of AV(i)
    pend = None  # (ut, f, h, qh, kp, ex, first, last)
    ut_cur = None
    for it_idx, (f, h, qh, kp, first, last) in enumerate(iters):
        if first:
            ut_cur = psU.tile([P, 512], F32, tag="U", bufs=2)
        ex = emit_scores_exp(f, h, qh, kp)
        if pend is not None:
            put, pf, ph, pqh, pkp, pex, pfirst, plast = pend
            emit_av(put, pf, ph, pkp, pex, pfirst, plast)
            if plast:  # finished a (h, qh) block: normalize it
                emit_norm(put, pf, ph, pqh)
        pend = (ut_cur, f, h, qh, kp, ex, first, last)
        # drain one O-proj group every other iteration
        if it_idx % 2 == 0 and oproj_queue:
            emit_oproj_group(*oproj_queue.pop(0))
        # after the first head of frame f, queue O-proj for frame f-1
        if last and h == 0 and qh == 1 and f >= 1:
            for tl in range(8):
                for g in range(3):
                    oproj_queue.append((f - 1, tl, g))
    put, pf, ph, pqh, pkp, pex, pfirst, plast = pend
    emit_av(put, pf, ph, pkp, pex, pfirst, plast)
    emit_norm(put, pf, ph, pqh)
    for item in oproj_queue:
        emit_oproj_group(*item)
    for tl in range(8):
        for g in range(3):
            emit_oproj_group(NUM_FRAMES - 1, tl, g)


def build_program():
    from concourse.bass_interp import get_hw_module

    nc = bacc.Bacc(
        "TRN2",
        target_bir_lowering=False,
        debug=False,
        enable_asserts=False,
        num_devices=N_CORES,
    )
    xt = nc.dram_tensor("xt", [D, N_SET], FP8, kind="ExternalInput").ap()
    wqk = nc.dram_tensor("wqk", [KC_D, P, 6, P], FP8, kind="ExternalInput").ap()
    wv = nc.dram_tensor("wv", [KC_D, P, C], FP8, kind="ExternalInput").ap()
    wo = nc.dram_tensor("wo", [3, P, D], FP8, kind="ExternalInput").ap()
    bqk = nc.dram_tensor("bqk", [P, 6], F32, kind="ExternalInput").ap()
    out = nc.dram_tensor("out", [N_SET, D], BF16, kind="ExternalOutput").ap()
    with tile.TileContext(nc) as tc:
        with ExitStack() as ctx:
            build_kernel_body(ctx, tc, xt, wqk, wv, wo, bqk, out)
    nc.finalize()
    nc.m = get_hw_module(nc.m)
    return nc


def _f32(a):
    return np.asarray(a, np.float32)


def make_in_maps(hidden_states, Wq, Wk, Wv, bq, bk, bv):
    """Per-core inputs. Core c = set (c//4), head group (c%4)."""
    hs = _f32(hidden_states).reshape(BF, S, D)
    Wq, Wk, Wv = _f32(Wq), _f32(Wk), _f32(Wv)
    bq, bk = _f32(bq), _f32(bk)
    xts = []
    for s in range(B):
        x = hs[s * NUM_FRAMES:(s + 1) * NUM_FRAMES].reshape(N_SET, D)
        xts.append(np.ascontiguousarray(x.T).astype(NPFP8))
    in_maps = []
    for cidx in range(N_CORES):
        s, g = cidx // GROUPS, cidx % GROUPS
        ghb = g * HG  # global head base
        # wqk: [KC_D, P, 6, P]; chunk g holds cols of the transposed proj
        wqk_a = np.zeros((D, 6, P), np.float32)
        bqk_a = np.zeros((P, 6), np.float32)
        for ch, (W, bias, heads) in enumerate([
            (Wq, bq, (0, 1)), (Wq, bq, (2, 3)),
            (Wk, bk, (0, 1)), (Wk, bk, (2, 3)),
            (Wq, bq, (4,)), (Wk, bk, (4,)),
        ]):
            for side, h in enumerate(heads):
                cols = slice((ghb + h) * HD, (ghb + h + 1) * HD)
                wqk_a[:, ch, side * HD:(side + 1) * HD] = W[:, cols]
                bqk_a[side * HD:(side + 1) * HD, ch] = bias[cols]
        wqk_a = (wqk_a * WQK_SC).reshape(KC_D, P, 6, P).astype(NPFP8)
        bqk_a = (bqk_a * WQK_SC).astype(np.float32)
        wv_a = (Wv[:, g * C:(g + 1) * C] * WV_SC).reshape(KC_D, P, C)
        in_maps.append({
            "xt": xts[s],
            "wqk": wqk_a,
            "wv": wv_a.astype(NPFP8),
            "bqk": bqk_a,
        })
    return in_maps


def make_wo_pad(Wo, g):
    """wo: [3, P, D] fp8 x16; ch c partition p = Wo row g*C + c*128 + p."""
    Wo = _f32(Wo)
    wo_a = np.zeros((3, P, D), np.float32)
    rows = Wo[g * C:(g + 1) * C, :] * WO_SC  # [320, D]
    wo_a[0] = rows[0:128]
    wo_a[1] = rows[128:256]
    wo_a[2, 0:64] = rows[256:320]
    return wo_a.astype(NPFP8)


def assemble_output(results, hidden_states, Wo, bo, bv):
    """Sum per-core partials, divide by OUT_DIV, add bo + bv@Wo + residual."""
    hs = _f32(hidden_states)
    bo_eff = _f32(bo) + _f32(bv) @ _f32(Wo)
    out = np.empty((BF, S, D), np.float32)
    for s in range(B):
        acc = np.zeros((N_SET, D), np.float32)
        for g in range(GROUPS):
            acc += _f32(results[s * GROUPS + g]["out"])
        out[s * NUM_FRAMES:(s + 1) * NUM_FRAMES] = (
            acc.reshape(NUM_FRAMES, S, D) / OUT_DIV
            + bo_eff[None, None, :]
            + hs[s * NUM_FRAMES:(s + 1) * NUM_FRAMES]
        )
    return out


_PROGRAM = None


def kernel(hidden_states, Wq, Wk, Wv, Wo, bq, bk, bv, bo):
    global _PROGRAM
    if _PROGRAM is None:
        _PROGRAM = build_program()
    nc = _PROGRAM

    in_maps = make_in_maps(hidden_states, Wq, Wk, Wv, bq, bk, bv)
    for c in range(N_CORES):
        in_maps[c]["wo"] = make_wo_pad(Wo, c % GROUPS)

    res = bass_utils.run_bass_kernel_spmd(nc, in_maps,
                                          core_ids=list(range(N_CORES)))
    return assemble_output(res.results, hidden_states, Wo, bo, bv)


# revision 8
# speedup vs baseline: 2.0266x; 1.4737x over previous
"""Trainium2 Bass kernel for nn_ConsistentSelfAttentionProcessor (v2, fp8).

Reference computation (per frame-set of NUM_FRAMES=4 frames):
    q,k,v = hs@Wq+bq, hs@Wk+bk, hs@Wv+bv          # [BF,S,D]
    per head: K_comb = [K(frame0_of_set); K(own)]  # 2S keys
    out = softmax(q@K_comb^T/sqrt(hd)) @ V_comb @ Wo + bo + hs

Sharding: 8 cores = 2 frame-sets x 4 head-groups of 5 heads.
Core computes a partial  attn(set, heads_g) @ Wo[rows_g]  in bf16 (x4096);
host sums the 4 per-set partials, divides by 4096, adds bo + bv@Wo + residual.

v2 design (vs bf16 v1):
 - everything on the PE in fp8e4 (weights host-scaled x64 q/k/v, x16 Wo so
   values sit in fp8 normal range; scales divided out on the host)
 - q^T/k^T produced directly transposed (W stationary, x^T moving): no PE
   transposes.  2 heads packed per 128-partition chunk (parity rows).
 - DoubleRow fp8 matmuls (2x128 contraction per pass) for projections and AV.
 - V stationary blocks are [v(64)|ones(64)] (even heads) / [ones|v] (odd), so
   the AV matmul emits 64 broadcast copies of the softmax denominator next to
   the 64 attn rows: normalization needs no PE/DMA broadcast.
 - softmax exp split between ACT (hardware Exp->fp8) and DVE (Schraudolph
   int8 bit-trick writing fp8e4 bit patterns directly).
 - denominator reciprocal via int32 bit-trick on DVE (no 6.5us RECIPROCAL),
   row-shifted to the attn partitions with a tiny SBUF->SBUF DMA.
 - scores software-pipelined one iteration ahead of AV so the PE never waits
   on exp.

Frame 0 of each set attends to [K0;K0] == softmax over K0: uses 4 key-chunk
pairs instead of 8.
"""

import sys
from contextlib import ExitStack

import numpy as np

sys.path.insert(0, "/opt/trn_rl_repo")

import ml_dtypes  # noqa: E402

import concourse.bass as bass  # noqa: E402
import concourse.mybir as mybir  # noqa: E402
import concourse.tile as tile  # noqa: E402
from concourse import bacc, bass_utils  # noqa: E402

FP8 = mybir.dt.float8e4
BF16 = mybir.dt.bfloat16
F32 = mybir.dt.float32
I8 = mybir.dt.int8
I32 = mybir.dt.int32
NPFP8 = ml_dtypes.float8_e4m3
NPBF16 = ml_dtypes.bfloat16
DR = mybir.MatmulPerfMode.DoubleRow
EXP = mybir.ActivationFunctionType.Exp
IDENT = mybir.ActivationFunctionType.Identity
MULT = mybir.AluOpType.mult
ADD = mybir.AluOpType.add

NUM_FRAMES = 4
HEADS = 20
BF, S, D = 8, 1024, 1280
HD = 64
B = BF // NUM_FRAMES
N_SET = NUM_FRAMES * S  # 4096
N_CORES = 8
GROUPS = 4
HG = HEADS // GROUPS  # 5
C = HG * HD  # 320
P = 128
KC_D = D // P  # 10

# host-side scalings (divided out on host)
WQK_SC = 32.0
WV_SC = 32.0
WO_SC = 16.0
ATN_SC = 8.0  # applied in the normalize mult
OUT_DIV = WV_SC * ATN_SC * WO_SC  # 4096

SCALE_EXP = 0.125 / (WQK_SC * WQK_SC)  # exp(score_dev * SCALE_EXP)
SCH_A = (8.0 / np.log(2.0)) * SCALE_EXP
SCH_B = 55.5
RMAGIC = 0x7EF0FFFF

# qkt chunk map: 6 chunks of [low(0:64) | high(64:128)] rows
#   ch0=[q0|q1] ch1=[q2|q3] ch2=[k0|k1] ch3=[k2|k3] ch4=[q4|-] ch5=[k4|-]
QCH = [0, 0, 1, 1, 4]
KCH = [2, 2, 3, 3, 5]
HROW = [0, 64, 0, 64, 0]  # base row (parity) of head h


def build_kernel_body(ctx: ExitStack, tc: tile.TileContext, xt, wqk, wv, wo,
                      bqk, out):
    """Per-core program.

    xt:   [D, N_SET]          fp8  (X^T for this set)
    wqk:  [KC_D, P, 6, P]     fp8  (transposed-proj stationaries, x64)
    wv:   [KC_D, P, C]        fp8  (x64)
    wo:   [3, P, D]           fp8  (O-proj moving, x16; ch2 rows 64:128 zero)
    bqk:  [P, 6]              f32  (per-partition q/k biases, x64)
    out:  [N_SET, D]          bf16 (partial output x4096, unsummed)
    """
    nc = tc.nc

    const = ctx.enter_context(tc.tile_pool(name="const", bufs=1))
    persist = ctx.enter_context(tc.tile_pool(name="persist", bufs=1))
    work = ctx.enter_context(tc.tile_pool(name="work", bufs=3))
    psA = ctx.enter_context(tc.tile_pool(name="psA", bufs=2, space="PSUM"))
    psU = ctx.enter_context(tc.tile_pool(name="psU", bufs=2, space="PSUM"))
    psO = ctx.enter_context(tc.tile_pool(name="psO", bufs=2, space="PSUM"))

    # ---- constant loads -----------------------------------------------------
    xt_sb = const.tile([P, KC_D, N_SET], FP8, tag="xt")
    for q in range(4):
        nc.sync.dma_start(
            xt_sb[:, :, q * 1024:(q + 1) * 1024],
            xt[:, q * 1024:(q + 1) * 1024].rearrange("(c p) n -> p c n", p=P),
        )
    wqk_sb = const.tile([P, KC_D, 6, P], FP8, tag="wqk")
    nc.sync.dma_start(wqk_sb, wqk.rearrange("c p g m -> p c g m"))
    wv_sb = const.tile([P, KC_D, C], FP8, tag="wv")
    nc.sync.dma_start(wv_sb, wv.rearrange("c p n -> p c n"))
    wo_sb = const.tile([P, 3, D], FP8, tag="wo")
    nc.sync.dma_start(wo_sb, wo.rearrange("c p n -> p c n"))
    bqk_sb = const.tile([P, 6], F32, tag="bqk")
    nc.sync.dma_start(bqk_sb, bqk)

    # ---- persistent intermediates ------------------------------------------
    qkt = persist.tile([P, 6, N_SET], FP8, tag="qkt")
    # vsb blocks per head: even [v|ones], odd [ones|v]
    vsb = persist.tile([P, N_SET // P, HG, P], FP8, tag="vsb")
    nc.gpsimd.memset(vsb[:, :, 0::2, HD:P], 1.0)
    nc.gpsimd.memset(vsb[:, :, 1::2, 0:HD], 1.0)
    atn_f = [
        persist.tile([P, 3, S], FP8, tag=f"atn{f}", name=f"atn{f}")
        for f in range(NUM_FRAMES)
    ]

    copy_tog = [0]

    def psum_to_sbuf(dst, src, bias=None):
        """Alternate psum->sbuf copies between ACT and DVE."""
        copy_tog[0] ^= 1
        if copy_tog[0]:
            if bias is not None:
                nc.scalar.activation(dst, src, IDENT, bias=bias, scale=1.0)
            else:
                nc.scalar.copy(dst, src)
        else:
            if bias is not None:
                nc.vector.tensor_scalar(dst, src, bias, None, ADD)
            else:
                nc.vector.tensor_copy(dst, src)

    # ---- emitters -----------------------------------------------------------
    def emit_qkt_group(ts, g):
        """Transposed q/k projection: one psum group (5 DR matmuls + copy)."""
        tok = slice(ts * 512, (ts + 1) * 512)
        pq = psO.tile([P, 512], F32, tag="O", bufs=2, name="pq")
        for i in range(KC_D // 2):
            nc.tensor.matmul(
                pq, wqk_sb[:, 2 * i:2 * i + 2, g, :],
                xt_sb[:, 2 * i:2 * i + 2, tok],
                perf_mode=DR, start=(i == 0), stop=(i == KC_D // 2 - 1),
            )
        rows = slice(0, P) if g < 4 else slice(0, HD)
        psum_to_sbuf(qkt[rows, g, tok], pq[rows, :],
                     bias=bqk_sb[rows, g:g + 1])

    def emit_v_chunk(t):
        """V projection for one 128-token chunk (5 DR matmuls + 2 copies)."""
        pv = psO.tile([P, 512], F32, tag="O", bufs=2, name="pv")
        for i in range(KC_D // 2):
            nc.tensor.matmul(
                pv[:, 0:C],
                xt_sb[:, 2 * i:2 * i + 2, t * P:(t + 1) * P],
                wv_sb[:, 2 * i:2 * i + 2, :],
                perf_mode=DR, start=(i == 0), stop=(i == KC_D // 2 - 1),
            )
        pvh = pv[:, 0:C].rearrange("p (h d) -> p h d", d=HD)
        psum_to_sbuf(vsb[:, t, 0::2, 0:HD], pvh[:, 0::2, :])
        psum_to_sbuf(vsb[:, t, 1::2, HD:P], pvh[:, 1::2, :])

    # exp engine pattern: 5 ACT : 3 DVE, DVE spread out
    EXP_PAT = [0, 1, 0, 1, 0, 0, 1, 0]
    exp_i = [0]

    def emit_scores_exp(f, h, qh, kp):
        b0 = HROW[h]
        qspan = slice(f * S + qh * 512, f * S + (qh + 1) * 512)
        sc = psA.tile([P, 2, 512], F32, tag="A", bufs=2, name="sc")
        for c in range(2):
            if kp < 4:
                kt = (2 * kp + c) * P  # ref frame (set tokens 0:1024)
            else:
                kt = f * S + (2 * (kp - 4) + c) * P
            nc.tensor.matmul(
                sc[:, c, :],
                qkt[b0:b0 + HD, KCH[h], kt:kt + P],
                qkt[b0:b0 + HD, QCH[h], qspan],
            )
        ex = work.tile([P, 2, 512], FP8, tag="ex", bufs=6, name="ex")
        sel = EXP_PAT[exp_i[0] % 8]
        exp_i[0] += 1
        if sel == 0:
            nc.scalar.activation(ex, sc, EXP, scale=SCALE_EXP)
        else:
            nc.vector.tensor_scalar(ex.bitcast(I8), sc, SCH_A, SCH_B,
                                    MULT, ADD)
        return ex

    def emit_av(ut, f, h, kp, ex, first, last):
        tp = 2 * kp if kp < 4 else f * 8 + 2 * (kp - 4)
        nc.tensor.matmul(
            ut, vsb[:, tp:tp + 2, h, :], ex,
            perf_mode=DR, start=first, stop=last,
        )

    def emit_norm(ut, f, h, qh):
        """atn[rows, ch, qh*512:] = (ut[rows]*ATN_SC) * (1/denom)."""
        if h % 2 == 0:
            arows, drows = slice(0, HD), slice(HD, P)
        else:
            arows, drows = slice(HD, P), slice(0, HD)
        rc1 = work.tile([P, 512], F32, tag="rc1", bufs=3, name="rc1")
        nc.vector.tensor_scalar(rc1[drows, :].bitcast(I32),
                                ut[drows, :].bitcast(I32),
                                -1, RMAGIC, MULT, ADD)
        rc2 = work.tile([P, 512], F32, tag="rc2", bufs=3, name="rc2")
        nc.sync.dma_start(rc2[arows, :], rc1[drows, :])
        nc.vector.scalar_tensor_tensor(
            atn_f[f][arows, h // 2, qh * 512:(qh + 1) * 512],
            ut[arows, :], ATN_SC, rc2[arows, :], MULT, MULT,
        )

    ou_state = {}

    def emit_oproj_group(f, tl, g):
        t = f * 8 + tl
        n0 = g * 512
        nw = 512 if g < 2 else 256
        po = psO.tile([P, 512], F32, tag="O", bufs=2, name="po")
        nc.tensor.matmul(
            po[:, 0:nw], atn_f[f][:, 0:2, tl * P:(tl + 1) * P],
            wo_sb[:, 0:2, n0:n0 + nw],
            perf_mode=DR, start=True, stop=False,
        )
        nc.tensor.matmul(
            po[:, 0:nw], atn_f[f][0:HD, 2, tl * P:(tl + 1) * P],
            wo_sb[0:HD, 2, n0:n0 + nw],
            start=False, stop=True,
        )
        if g == 0:
            ou_state[t] = work.tile([P, D], BF16, tag="ou", bufs=3,
                                    name=f"ou{t}")
        psum_to_sbuf(ou_state[t][:, n0:n0 + nw], po[:, 0:nw])
        if g == 2:
            nc.sync.dma_start(out[t * P:(t + 1) * P, :], ou_state.pop(t))

    # ---- phase 1 upfront: slabs 0,1 (dense PE; ramps the p-state) ----------
    for s in (0, 1):
        for g in range(6):
            emit_qkt_group(s, g)
        for tl in range(4):
            emit_v_chunk(s * 4 + tl)

    # production queue for slabs 2..7 + O-proj queue
    prod_q = [(s, ('qkt', s, g)) for s in range(2, 8) for g in range(6)]
    prod_q += [(s, ('v', s * 4 + tl)) for s in range(2, 8) for tl in range(4)]
    prod_q.sort(key=lambda e: e[0])
    oproj_q = []

    def run_filler(job):
        if job[0] == 'qkt':
            emit_qkt_group(job[1], job[2])
        elif job[0] == 'v':
            emit_v_chunk(job[1])
        else:
            emit_oproj_group(job[1], job[2], job[3])

    def pop_filler():
        if prod_q:
            run_filler(prod_q.pop(0)[1])
        elif oproj_q:
            run_filler(oproj_q.pop(0))

    # ---- attention stream (AV lagged 2 iters behind scores) ----------------
    # frames 0-2: blocks (h, qh); frame 3: qh-outer so O-proj(3, tl<4) can
    # start before qh=1 finishes
    stream = []
    for f in range(NUM_FRAMES):
        nkp = 4 if f == 0 else 8
        stream.append(('drain', 2 * f + 1))
        blocks = ([(h, qh) for h in range(HG) for qh in range(2)]
                  if f < 3 else
                  [(h, qh) for qh in range(2) for h in range(HG)])
        for bi, (h, qh) in enumerate(blocks):
            if bi == 1 and f >= 1:
                stream.append(('oproj', f - 1, range(8)))
            if f == 3 and bi == 6:
                stream.append(('oproj', 3, range(4)))
            for kp in range(nkp):
                stream.append(('it', f, h, qh, kp, kp == 0, kp == nkp - 1))

    pend = []
    ut_cur = None
    it_i = 0
    for ev in stream:
        if ev[0] == 'drain':
            while prod_q and prod_q[0][0] <= ev[1]:
                run_filler(prod_q.pop(0)[1])
            continue
        if ev[0] == 'oproj':
            for tl in ev[2]:
                for g in range(3):
                    oproj_q.append(('oproj', ev[1], tl, g))
            continue
        _, f, h, qh, kp, first, last = ev
        if first:
            ut_cur = psU.tile([P, 512], F32, tag="U", bufs=2, name="ut")
        ex = emit_scores_exp(f, h, qh, kp)
        pend.append((ut_cur, f, h, qh, kp, ex, first, last))
        if len(pend) > 2:
            put, pf, ph, pqh, pkp, pex, pfirst, plast = pend.pop(0)
            emit_av(put, pf, ph, pkp, pex, pfirst, plast)
            if plast:
                emit_norm(put, pf, ph, pqh)
        if it_i % 2 == 0:
            pop_filler()
        it_i += 1
    for put, pf, ph, pqh, pkp, pex, pfirst, plast in pend:
        emit_av(put, pf, ph, pkp, pex, pfirst, plast)
        if plast:
            emit_norm(put, pf, ph, pqh)
    # tail: remaining O-proj (frame 3 wave B)
    for tl in range(4, 8):
        for g in range(3):
            oproj_q.append(('oproj', 3, tl, g))
    while prod_q or oproj_q:
        pop_filler()


def build_program():
    from concourse.bass_interp import get_hw_module

    nc = bacc.Bacc(
        "TRN2",
        target_bir_lowering=False,
        debug=False,
        enable_asserts=False,
        num_devices=N_CORES,
    )
    xt = nc.dram_tensor("xt", [D, N_SET], FP8, kind="ExternalInput").ap()
    wqk = nc.dram_tensor("wqk", [KC_D, P, 6, P], FP8, kind="ExternalInput").ap()
    wv = nc.dram_tensor("wv", [KC_D, P, C], FP8, kind="ExternalInput").ap()
    wo = nc.dram_tensor("wo", [3, P, D], FP8, kind="ExternalInput").ap()
    bqk = nc.dram_tensor("bqk", [P, 6], F32, kind="ExternalInput").ap()
    out = nc.dram_tensor("out", [N_SET, D], BF16, kind="ExternalOutput").ap()
    with tile.TileContext(nc) as tc:
        with ExitStack() as ctx:
            build_kernel_body(ctx, tc, xt, wqk, wv, wo, bqk, out)
    nc.finalize()
    nc.m = get_hw_module(nc.m)
    return nc


def _f32(a):
    return np.asarray(a, np.float32)


def make_in_maps(hidden_states, Wq, Wk, Wv, bq, bk, bv):
    """Per-core inputs. Core c = set (c//4), head group (c%4)."""
    hs = _f32(hidden_states).reshape(BF, S, D)
    Wq, Wk, Wv = _f32(Wq), _f32(Wk), _f32(Wv)
    bq, bk = _f32(bq), _f32(bk)
    xts = []
    for s in range(B):
        x = hs[s * NUM_FRAMES:(s + 1) * NUM_FRAMES].reshape(N_SET, D)
        xts.append(np.ascontiguousarray(x.T).astype(NPFP8))
    in_maps = []
    for cidx in range(N_CORES):
        s, g = cidx // GROUPS, cidx % GROUPS
        ghb = g * HG  # global head base
        # wqk: [KC_D, P, 6, P]; chunk g holds cols of the transposed proj
        wqk_a = np.zeros((D, 6, P), np.float32)
        bqk_a = np.zeros((P, 6), np.float32)
        for ch, (W, bias, heads) in enumerate([
            (Wq, bq, (0, 1)), (Wq, bq, (2, 3)),
            (Wk, bk, (0, 1)), (Wk, bk, (2, 3)),
            (Wq, bq, (4,)), (Wk, bk, (4,)),
        ]):
            for side, h in enumerate(heads):
                cols = slice((ghb + h) * HD, (ghb + h + 1) * HD)
                wqk_a[:, ch, side * HD:(side + 1) * HD] = W[:, cols]
                bqk_a[side * HD:(side + 1) * HD, ch] = bias[cols]
        wqk_a = (wqk_a * WQK_SC).reshape(KC_D, P, 6, P).astype(NPFP8)
        bqk_a = (bqk_a * WQK_SC).astype(np.float32)
        wv_a = (Wv[:, g * C:(g + 1) * C] * WV_SC).reshape(KC_D, P, C)
        in_maps.append({
            "xt": xts[s],
            "wqk": wqk_a,
            "wv": wv_a.astype(NPFP8),
            "bqk": bqk_a,
        })
    return in_maps


def make_wo_pad(Wo, g):
    """wo: [3, P, D] fp8 x16; ch c partition p = Wo row g*C + c*128 + p."""
    Wo = _f32(Wo)
    wo_a = np.zeros((3, P, D), np.float32)
    rows = Wo[g * C:(g + 1) * C, :] * WO_SC  # [320, D]
    wo_a[0] = rows[0:128]
    wo_a[1] = rows[128:256]
    wo_a[2, 0:64] = rows[256:320]
    return wo_a.astype(NPFP8)


def assemble_output(results, hidden_states, Wo, bo, bv):
    """Sum per-core partials, divide by OUT_DIV, add bo + bv@Wo + residual."""
    hs = _f32(hidden_states)
    bo_eff = _f32(bo) + _f32(bv) @ _f32(Wo)
    out = np.empty((BF, S, D), np.float32)
    for s in range(B):
        acc = np.zeros((N_SET, D), np.float32)
        for g in range(GROUPS):
            acc += _f32(results[s * GROUPS + g]["out"])
        out[s * NUM_FRAMES:(s + 1) * NUM_FRAMES] = (
            acc.reshape(NUM_FRAMES, S, D) / OUT_DIV
            + bo_eff[None, None, :]
            + hs[s * NUM_FRAMES:(s + 1) * NUM_FRAMES]
        )
    return out


_PROGRAM = None


def kernel(hidden_states, Wq, Wk, Wv, Wo, bq, bk, bv, bo):
    global _PROGRAM
    if _PROGRAM is None:
        _PROGRAM = build_program()
    nc = _PROGRAM

    in_maps = make_in_maps(hidden_states, Wq, Wk, Wv, bq, bk, bv)
    for c in range(N_CORES):
        in_maps[c]["wo"] = make_wo_pad(Wo, c % GROUPS)

    res = bass_utils.run_bass_kernel_spmd(nc, in_maps,
                                          core_ids=list(range(N_CORES)))
    return assemble_output(res.results, hidden_states, Wo, bo, bv)
